# revision 1
# baseline (speedup 1.0000x reference)
"""NemotronH Top-k MoE router on 8 Trainium2 NeuronCores.

Strategy
--------
Token-parallel: 16384 tokens are sharded 2048-per-core across 8 cores;
the router weight [256, 4096] and bias are replicated. No collectives.

Matmul (the memory/compute-dominant part): logits = hidden @ weight.T
needs full fp32 accuracy (top-k selection margins are ~1e-6), but the
PE's native fp32 matmul is ~8x slower than fp16. So hidden and weight
are split host-side into fp16 hi/lo planes (x = h + l/2048, 22 mantissa
bits) and the product is computed in three fp16 passes accumulated in
fp32 PSUM:  logits = h@u + (h@v + l@u)/2048, dropping the O(2^-22) l@v
term. Host also pre-transposes to [H, T] so the contraction dim lands
on SBUF partitions with fully-contiguous DMA. Subnormal fp16 values are
flushed to zero host-side so PE FTZ behavior cannot skew the split.

Routing per 128-token tile (t on partitions, experts on free axis):
PE-transpose of the [e, t] logits, ACT sigmoid, then DVE/GPSIMD ops:
group top-2 via reduce_max + match_replace, group top-4 via max8
threshold, top-8 via max8 + max_index, per-slot score extraction via
is_equal + accumulate (no per-partition gather exists), then normalize.
"""
import sys
sys.path.insert(0, "/opt/trn_rl_repo")

import numpy as np

from concourse import bacc, tile, mybir
from concourse.bass_utils import run_bass_kernel_spmd

F32 = mybir.dt.float32
F16 = mybir.dt.float16
U16 = mybir.dt.uint16
I32 = mybir.dt.int32
AF = mybir.ActivationFunctionType
ALU = mybir.AluOpType

T_TOTAL = 16384
H = 4096
E = 256
G, GS = 8, 32
TOP_K = 8
N_CORES = 8
T_CORE = T_TOTAL // N_CORES      # 2048
TT = 512                         # tokens per tile
N_TILES = T_CORE // TT           # 4
KC = H // 128                    # 32 k-chunks
KH = KC // 2                     # 16 per k-half
S = 2048.0                       # lo-plane scale (2^11)
ROUTED_SCALING = 2.5


def build_program(reps=1):
    nc = bacc.Bacc("TRN2", target_bir_lowering=False)
    hst_h = nc.dram_tensor("hst_h", [H, T_CORE], F16, kind="ExternalInput")
    hst_l = nc.dram_tensor("hst_l", [H, T_CORE], F16, kind="ExternalInput")
    wt_h = nc.dram_tensor("wt_h", [H, E], F16, kind="ExternalInput")
    wt_l = nc.dram_tensor("wt_l", [H, E], F16, kind="ExternalInput")
    bias_d = nc.dram_tensor("bias_bc", [128, E], F32, kind="ExternalInput")
    iota_d = nc.dram_tensor("iota_bc", [128, E], F32, kind="ExternalInput")
    ident_d = nc.dram_tensor("ident", [128, 128], F32, kind="ExternalInput")
    idx_out = nc.dram_tensor("idx_out", [T_CORE, TOP_K], I32, kind="ExternalOutput")
    w_out = nc.dram_tensor("w_out", [T_CORE, TOP_K], F32, kind="ExternalOutput")

    with tile.TileContext(nc) as tc:
        with (
            tc.tile_pool(name="const", bufs=1) as cpool,
            tc.tile_pool(name="hs", bufs=2) as hspool,
            tc.tile_pool(name="comb", bufs=2) as combpool,
            tc.tile_pool(name="rt", bufs=2) as rt,
            tc.tile_pool(name="outp", bufs=2) as outp,
            tc.tile_pool(name="ps", bufs=2, space="PSUM") as ps,
            tc.tile_pool(name="pslg", bufs=2, space="PSUM") as pslg,
        ):
            wh_t = cpool.tile([128, KC, E], F16)
            wl_t = cpool.tile([128, KC, E], F16)
            bias_t = cpool.tile([128, E], F32)
            iota_t = cpool.tile([128, E], F32)
            ident_t = cpool.tile([128, 128], F32)
            nc.sync.dma_start(wh_t[:], wt_h.rearrange("(c p) e -> p c e", p=128))
            nc.sync.dma_start(wl_t[:], wt_l.rearrange("(c p) e -> p c e", p=128))
            nc.sync.dma_start(bias_t[:], bias_d[:])
            nc.sync.dma_start(iota_t[:], iota_d[:])
            nc.sync.dma_start(ident_t[:], ident_d[:])

            def body():
                for it in range(N_TILES):
                    t0 = it * TT
                    # ---- load hidden planes, split in k-halves for pipelining
                    hh = [hspool.tile([128, KH, TT], F16, tag=f"hh{k}", name=f"hh{k}")
                          for k in range(2)]
                    hl = [hspool.tile([128, KH, TT], F16, tag=f"hl{k}", name=f"hl{k}")
                          for k in range(2)]
                    for k in range(2):
                        src_h = hst_h[k * KH * 128:(k + 1) * KH * 128, t0:t0 + TT]
                        src_l = hst_l[k * KH * 128:(k + 1) * KH * 128, t0:t0 + TT]
                        nc.sync.dma_start(hh[k][:], src_h.rearrange("(c p) t -> p c t", p=128))
                        nc.sync.dma_start(hl[k][:], src_l.rearrange("(c p) t -> p c t", p=128))

                    # ---- 3-pass matmul: [e_half, t] psum accumulation
                    comb = combpool.tile([128, 2, TT], F32, tag="comb")
                    for eh in range(2):
                        es = slice(eh * 128, (eh + 1) * 128)
                        main = ps.tile([128, TT], F32, tag="main")
                        corr = ps.tile([128, TT], F32, tag="corr")
                        for c in range(KC):
                            k, ci = divmod(c, KH)
                            nc.tensor.matmul(
                                main[:], wh_t[:, c, es], hh[k][:, ci, :],
                                start=(c == 0), stop=(c == KC - 1))
                        for c in range(KC):
                            k, ci = divmod(c, KH)
                            nc.tensor.matmul(
                                corr[:], wl_t[:, c, es], hh[k][:, ci, :],
                                start=(c == 0), stop=False)
                            nc.tensor.matmul(
                                corr[:], wh_t[:, c, es], hl[k][:, ci, :],
                                start=False, stop=(c == KC - 1))
                        # comb = main + corr/S (two steps: only one PSUM read per op)
                        corr_s = combpool.tile([128, TT], F32, tag="corr_s")
                        nc.scalar.activation(corr_s[:], corr[:], AF.Copy, scale=1.0 / S)
                        nc.vector.tensor_tensor(comb[:, eh, :], corr_s[:], main[:], ALU.add)

                    # ---- routing per 128-token subtile
                    iouts = outp.tile([128, TT // 128, TOP_K], I32, tag="iouts")
                    wouts = outp.tile([128, TT // 128, TOP_K], F32, tag="wouts")
                    for sub in range(TT // 128):
                        ss = slice(sub * 128, (sub + 1) * 128)
                        lgps = pslg.tile([128, E], F32, tag="lg")
                        nc.tensor.transpose(lgps[:, 0:128], comb[:, 0, ss], ident_t[:])
                        nc.tensor.transpose(lgps[:, 128:256], comb[:, 1, ss], ident_t[:])

                        scores = rt.tile([128, E], F32, tag="scores")
                        nc.scalar.activation(scores[:], lgps[:], AF.Sigmoid)

                        s4c = rt.tile([128, E], F32, tag="s4c")
                        nc.gpsimd.tensor_tensor(s4c[:], scores[:], bias_t[:], ALU.add)

                        m1 = rt.tile([128, G], F32, tag="m1")
                        nc.vector.reduce_max(
                            m1[:], s4c[:].rearrange("p (g s) -> p g s", g=G),
                            axis=mybir.AxisListType.X)
                        s4cr = rt.tile([128, E], F32, tag="s4cr")
                        nc.vector.match_replace(s4cr[:], m1[:], s4c[:], -1e30)
                        m2 = rt.tile([128, G], F32, tag="m2")
                        nc.vector.reduce_max(
                            m2[:], s4cr[:].rearrange("p (g s) -> p g s", g=G),
                            axis=mybir.AxisListType.X)
                        gsc = rt.tile([128, G], F32, tag="gsc")
                        nc.vector.tensor_tensor(gsc[:], m1[:], m2[:], ALU.add)

                        gsorted = rt.tile([128, 8], F32, tag="gsorted")
                        nc.vector.max(gsorted[:], gsc[:])
                        gmask = rt.tile([128, G], F32, tag="gmask")
                        nc.vector.tensor_scalar(
                            gmask[:], gsc[:], gsorted[:, 3:4], None, ALU.is_ge)

                        masked = rt.tile([128, E], F32, tag="masked")
                        nc.gpsimd.tensor_tensor(
                            masked[:].rearrange("p (g s) -> p g s", g=G),
                            s4c[:].rearrange("p (g s) -> p g s", g=G),
                            gmask[:].unsqueeze(-1).broadcast_to([128, G, GS]),
                            ALU.mult)

                        vals = rt.tile([128, 8], F32, tag="vals")
                        nc.vector.max(vals[:], masked[:])
                        idx16 = rt.tile([128, 8], U16, tag="idx16")
                        nc.vector.max_index(idx16[:], vals[:], masked[:])

                        # per-slot score extraction: w8[k] = sum(scores * (masked == vals[k]))
                        # per-slot gather scores[idx[k]]: match idx against an iota
                        # row (unique values -> tie-safe), accumulate the product
                        idxf = rt.tile([128, 8], F32, tag="idxf")
                        nc.vector.tensor_copy(idxf[:], idx16[:])
                        w8 = rt.tile([128, 8], F32, tag="w8")
                        scratch = rt.tile([128, E], F32, tag="scratch")
                        for k in range(TOP_K):
                            nc.vector.scalar_tensor_tensor(
                                scratch[:], iota_t[:], idxf[:, k:k + 1], scores[:],
                                ALU.is_equal, ALU.mult,
                                accum_out=w8[:, k:k + 1])

                        denom = rt.tile([128, 1], F32, tag="denom")
                        nc.vector.reduce_sum(denom[:], w8[:], axis=mybir.AxisListType.X)
                        rec = rt.tile([128, 1], F32, tag="rec")
                        nc.vector.tensor_scalar_add(denom[:], denom[:], 1e-20)
                        nc.vector.reciprocal(rec[:], denom[:])
                        nc.vector.tensor_scalar_mul(rec[:], rec[:], ROUTED_SCALING)

                        nc.vector.tensor_scalar(
                            wouts[:, sub, :], w8[:], rec[:, 0:1], None, ALU.mult)
                        nc.vector.tensor_copy(iouts[:, sub, :], idx16[:])

                    nc.sync.dma_start(
                        idx_out[t0:t0 + TT, :].rearrange("(s p) k -> p s k", p=128),
                        iouts[:])
                    nc.sync.dma_start(
                        w_out[t0:t0 + TT, :].rearrange("(s p) k -> p s k", p=128),
                        wouts[:])

            if reps == 1:
                body()
            else:
                with tc.For_i(0, reps, 1):
                    body()
    nc.compile()
    return nc


_PROGRAM_CACHE = {}


def _get_program(reps=1):
    if reps not in _PROGRAM_CACHE:
        _PROGRAM_CACHE[reps] = build_program(reps)
    return _PROGRAM_CACHE[reps]


_F16_MIN_NORMAL = 2.0 ** -14


def _split_f16(x):
    """x (f32) -> (h, l) fp16 planes with x ~= h + l/S; subnormals zeroed."""
    h = x.astype(np.float16)
    h32 = h.astype(np.float32)
    h = np.where(np.abs(h32) < _F16_MIN_NORMAL, np.float16(0), h)
    h32 = h.astype(np.float32)
    l = ((x - h32) * np.float32(S)).astype(np.float16)
    l32 = l.astype(np.float32)
    l = np.where(np.abs(l32) < _F16_MIN_NORMAL, np.float16(0), l)
    return h, l


def _prepare_inputs(hidden_states, weight, e_score_correction_bias):
    hs = np.asarray(hidden_states, dtype=np.float32)
    w = np.asarray(weight, dtype=np.float32)
    b = np.asarray(e_score_correction_bias, dtype=np.float32)

    wh, wl = _split_f16(w)
    wt_h = np.ascontiguousarray(wh.T)        # [H, E]
    wt_l = np.ascontiguousarray(wl.T)
    bias_bc = np.ascontiguousarray(np.broadcast_to(b, (128, E)))
    iota_bc = np.ascontiguousarray(
        np.broadcast_to(np.arange(E, dtype=np.float32), (128, E)))
    ident = np.eye(128, dtype=np.float32)

    in_maps = []
    for c in range(N_CORES):
        sl = hs[c * T_CORE:(c + 1) * T_CORE]  # [T_CORE, H]
        h, l = _split_f16(sl)
        in_maps.append({
            "hst_h": np.ascontiguousarray(h.T),
            "hst_l": np.ascontiguousarray(l.T),
            "wt_h": wt_h,
            "wt_l": wt_l,
            "bias_bc": bias_bc,
            "iota_bc": iota_bc,
            "ident": ident,
        })
    return in_maps


def kernel(hidden_states, weight, e_score_correction_bias):
    in_maps = _prepare_inputs(hidden_states, weight, e_score_correction_bias)
    nc = _get_program(1)
    res = run_bass_kernel_spmd(nc, in_maps, list(range(N_CORES)))
    idx = np.concatenate([r["idx_out"] for r in res.results], axis=0)
    w = np.concatenate([r["w_out"] for r in res.results], axis=0)
    return idx.astype(np.int32), w.astype(np.float32)



# revision 2
# speedup vs baseline: 4.8965x; 4.8965x over previous
"""NemotronH Top-k MoE router on 8 Trainium2 NeuronCores — v2.

Token-parallel: 16384 tokens sharded 2048/core; router weight replicated.

Matmul in [t, e] layout: for each 128-token block, PSUM out[t=128, e=256]
accumulates over 32 k-chunks with the hidden chunk as the stationary
operand and the weights as the moving operand.  This puts tokens on PSUM
partitions directly — no PE transposes — and keeps the PE stream dense.

Precision: identical arithmetic to the proven 3-pass fp16 scheme
(hi/lo fp16 planes, x = h + l/2048, 22 mantissa bits; products in fp22,
fp32 PSUM accumulation in the same order), so results are bit-identical
to the baseline kernel that matched the reference exactly.

Routing per block (tokens on partitions, experts on free axis): sigmoid,
bias add, group top-2 via reduce_max + match_replace, group top-4 via
max8 threshold, top-8 via max8 + max_index, per-slot score extraction via
is_equal(iota) + accumulate, normalize, scale.
"""
import sys
sys.path.insert(0, "/opt/trn_rl_repo")

import numpy as np

from concourse import bacc, tile, mybir
from concourse.bass_utils import run_bass_kernel_spmd

F32 = mybir.dt.float32
F16 = mybir.dt.float16
U16 = mybir.dt.uint16
I32 = mybir.dt.int32
AF = mybir.ActivationFunctionType
ALU = mybir.AluOpType

T_TOTAL = 16384
H = 4096
E = 256
G, GS = 8, 32
TOP_K = 8
N_CORES = 8
T_CORE = T_TOTAL // N_CORES      # 2048
TB = 128                         # tokens per block (PSUM partition dim)
NB = T_CORE // TB                # 16 blocks
KC = H // 128                    # 32 k-chunks
KH = KC // 2                     # 16 per weight half (split for startup)
S = 2048.0                       # lo-plane scale (2^11)
ROUTED_SCALING = 2.5


def build_program(reps=1):
    nc = bacc.Bacc("TRN2", target_bir_lowering=False)
    # host pre-layout: [blk, p, c, tb] so each partition line is 8 KB contiguous
    hst_h = nc.dram_tensor("hst_h", [NB, 128, KC, TB], F16, kind="ExternalInput")
    hst_l = nc.dram_tensor("hst_l", [NB, 128, KC, TB], F16, kind="ExternalInput")
    # weights pre-layout [p, c, e], split in two c-halves for faster startup
    wt_h = nc.dram_tensor("wt_h", [128, KC, E], F16, kind="ExternalInput")
    wt_l = nc.dram_tensor("wt_l", [128, KC, E], F16, kind="ExternalInput")
    bias_d = nc.dram_tensor("bias_bc", [128, E], F32, kind="ExternalInput")
    iota_d = nc.dram_tensor("iota_bc", [128, E], F32, kind="ExternalInput")
    idx_out = nc.dram_tensor("idx_out", [T_CORE, TOP_K], I32, kind="ExternalOutput")
    w_out = nc.dram_tensor("w_out", [T_CORE, TOP_K], F32, kind="ExternalOutput")

    with tile.TileContext(nc) as tc:
        with (
            tc.tile_pool(name="const", bufs=1) as cpool,
            tc.tile_pool(name="hs", bufs=3) as hspool,
            tc.tile_pool(name="rt", bufs=2) as rt,
            tc.tile_pool(name="outp", bufs=2) as outp,
            tc.tile_pool(name="psA", bufs=2, space="PSUM") as psA,
            tc.tile_pool(name="psB", bufs=2, space="PSUM") as psB,
        ):
            # weights as two c-halves so first matmuls start after half a DMA
            wh_t = [cpool.tile([128, KH, E], F16, name=f"wh{i}") for i in range(2)]
            wl_t = [cpool.tile([128, KH, E], F16, name=f"wl{i}") for i in range(2)]
            bias_t = cpool.tile([128, E], F32)
            iota_t = cpool.tile([128, E], F32)
            # weights on the ACT HWDGE ring (hidden goes on the SP ring) in
            # quarter-slices so block 0's matmuls can start early
            QW = KH // 2
            for i in range(2):
                for q in range(2):
                    cs = slice(q * QW, (q + 1) * QW)
                    nc.scalar.dma_start(wh_t[i][:, cs, :],
                                        wt_h[:, i * KH + q * QW:i * KH + (q + 1) * QW, :])
            for i in range(2):
                for q in range(2):
                    cs = slice(q * QW, (q + 1) * QW)
                    nc.scalar.dma_start(wl_t[i][:, cs, :],
                                        wt_l[:, i * KH + q * QW:i * KH + (q + 1) * QW, :])
            nc.scalar.dma_start(bias_t[:], bias_d[:])
            nc.scalar.dma_start(iota_t[:], iota_d[:])

            def body():
                for og in range(NB // 4):           # output groups of 4 blocks
                    iouts = outp.tile([128, 4, TOP_K], I32, tag="iouts")
                    wouts = outp.tile([128, 4, TOP_K], F32, tag="wouts")
                    for sub in range(4):
                        blk = og * 4 + sub
                        xh = hspool.tile([128, KC, TB], F16, tag="xh", name=f"xh{blk}")
                        xl = hspool.tile([128, KC, TB], F16, tag="xl", name=f"xl{blk}")
                        nc.sync.dma_start(xh[:], hst_h[blk])
                        nc.sync.dma_start(xl[:], hst_l[blk])

                        main = psA.tile([128, 512], F32, tag="main")
                        corr = psB.tile([128, 512], F32, tag="corr")
                        # main pass: logits_hi[t, e] += xh_c.T @ wh_c
                        for c in range(KC):
                            k, ci = divmod(c, KH)
                            nc.tensor.matmul(
                                main[:, 0:E], xh[:, c, :], wh_t[k][:, ci, :],
                                start=(c == 0), stop=(c == KC - 1))
                        # correction: corr += xh_c.T @ wl_c + xl_c.T @ wh_c
                        # (same interleaved accumulation order as the baseline)
                        for c in range(KC):
                            k, ci = divmod(c, KH)
                            nc.tensor.matmul(
                                corr[:, 0:E], xh[:, c, :], wl_t[k][:, ci, :],
                                start=(c == 0), stop=False)
                            nc.tensor.matmul(
                                corr[:, 0:E], xl[:, c, :], wh_t[k][:, ci, :],
                                start=False, stop=(c == KC - 1))

                        # comb = main + corr/S (exact: 1/S is a power of two)
                        corr_s = rt.tile([128, E], F32, tag="corr_s")
                        nc.scalar.activation(corr_s[:], corr[:, 0:E], AF.Copy,
                                             scale=1.0 / S)
                        comb = rt.tile([128, E], F32, tag="comb")
                        nc.vector.tensor_tensor(comb[:], corr_s[:], main[:, 0:E],
                                                ALU.add)

                        scores = rt.tile([128, E], F32, tag="scores")
                        nc.scalar.activation(scores[:], comb[:], AF.Sigmoid)

                        s4c = rt.tile([128, E], F32, tag="s4c")
                        nc.gpsimd.tensor_tensor(s4c[:], scores[:], bias_t[:], ALU.add)

                        m1 = rt.tile([128, G], F32, tag="m1")
                        nc.vector.reduce_max(
                            m1[:], s4c[:].rearrange("p (g s) -> p g s", g=G),
                            axis=mybir.AxisListType.X)
                        s4cr = rt.tile([128, E], F32, tag="s4cr")
                        nc.vector.match_replace(s4cr[:], m1[:], s4c[:], -1e30)
                        m2 = rt.tile([128, G], F32, tag="m2")
                        nc.vector.reduce_max(
                            m2[:], s4cr[:].rearrange("p (g s) -> p g s", g=G),
                            axis=mybir.AxisListType.X)
                        gsc = rt.tile([128, G], F32, tag="gsc")
                        nc.vector.tensor_tensor(gsc[:], m1[:], m2[:], ALU.add)

                        gsorted = rt.tile([128, 8], F32, tag="gsorted")
                        nc.vector.max(gsorted[:], gsc[:])
                        gmask = rt.tile([128, G], F32, tag="gmask")
                        nc.vector.tensor_scalar(
                            gmask[:], gsc[:], gsorted[:, 3:4], None, ALU.is_ge)

                        masked = rt.tile([128, E], F32, tag="masked")
                        nc.gpsimd.tensor_tensor(
                            masked[:].rearrange("p (g s) -> p g s", g=G),
                            s4c[:].rearrange("p (g s) -> p g s", g=G),
                            gmask[:].unsqueeze(-1).broadcast_to([128, G, GS]),
                            ALU.mult)

                        vals = rt.tile([128, 8], F32, tag="vals")
                        nc.vector.max(vals[:], masked[:])
                        idx16 = rt.tile([128, 8], U16, tag="idx16")
                        nc.vector.max_index(idx16[:], vals[:], masked[:])

                        # per-slot gather scores[idx[k]]: match idx against an
                        # iota row (unique values -> tie-safe), accumulate
                        idxf = rt.tile([128, 8], F32, tag="idxf")
                        nc.vector.tensor_copy(idxf[:], idx16[:])
                        w8 = rt.tile([128, 8], F32, tag="w8")
                        scratch = rt.tile([128, E], F32, tag="scratch")
                        for k in range(TOP_K):
                            nc.vector.scalar_tensor_tensor(
                                scratch[:], iota_t[:], idxf[:, k:k + 1], scores[:],
                                ALU.is_equal, ALU.mult,
                                accum_out=w8[:, k:k + 1])

                        denom = rt.tile([128, 1], F32, tag="denom")
                        nc.vector.reduce_sum(denom[:], w8[:], axis=mybir.AxisListType.X)
                        rec = rt.tile([128, 1], F32, tag="rec")
                        nc.vector.tensor_scalar_add(denom[:], denom[:], 1e-20)
                        nc.vector.reciprocal(rec[:], denom[:])
                        nc.vector.tensor_scalar_mul(rec[:], rec[:], ROUTED_SCALING)

                        nc.vector.tensor_scalar(
                            wouts[:, sub, :], w8[:], rec[:, 0:1], None, ALU.mult)
                        nc.vector.tensor_copy(iouts[:, sub, :], idx16[:])

                    t0 = og * 4 * TB
                    nc.sync.dma_start(
                        idx_out[t0:t0 + 4 * TB, :].rearrange("(s p) k -> p s k", p=128),
                        iouts[:])
                    nc.sync.dma_start(
                        w_out[t0:t0 + 4 * TB, :].rearrange("(s p) k -> p s k", p=128),
                        wouts[:])

            if reps == 1:
                body()
            else:
                with tc.For_i(0, reps, 1):
                    body()
    nc.compile()
    return nc


_PROGRAM_CACHE = {}


def _get_program(reps=1):
    if reps not in _PROGRAM_CACHE:
        _PROGRAM_CACHE[reps] = build_program(reps)
    return _PROGRAM_CACHE[reps]


_F16_MIN_NORMAL = 2.0 ** -14


def _split_f16(x):
    """x (f32) -> (h, l) fp16 planes with x ~= h + l/S; subnormals zeroed."""
    h = x.astype(np.float16)
    h32 = h.astype(np.float32)
    h = np.where(np.abs(h32) < _F16_MIN_NORMAL, np.float16(0), h)
    h32 = h.astype(np.float32)
    l = ((x - h32) * np.float32(S)).astype(np.float16)
    l32 = l.astype(np.float32)
    l = np.where(np.abs(l32) < _F16_MIN_NORMAL, np.float16(0), l)
    return h, l


def _blockify(plane_t):
    """[H, T_CORE] -> [NB, 128, KC, TB] so per-block partition lines are contiguous."""
    # element (h, t): h = c*128 + p, t = blk*TB + tb -> out[blk, p, c, tb]
    a = plane_t.reshape(KC, 128, NB, TB)       # [c, p, blk, tb]
    return np.ascontiguousarray(a.transpose(2, 1, 0, 3))


def _prepare_inputs(hidden_states, weight, e_score_correction_bias):
    hs = np.asarray(hidden_states, dtype=np.float32)
    w = np.asarray(weight, dtype=np.float32)
    b = np.asarray(e_score_correction_bias, dtype=np.float32)

    wh, wl = _split_f16(w)
    # [p, c, e] layout
    wt_h = np.ascontiguousarray(wh.T.reshape(KC, 128, E).transpose(1, 0, 2))
    wt_l = np.ascontiguousarray(wl.T.reshape(KC, 128, E).transpose(1, 0, 2))
    bias_bc = np.ascontiguousarray(np.broadcast_to(b, (128, E)))
    iota_bc = np.ascontiguousarray(
        np.broadcast_to(np.arange(E, dtype=np.float32), (128, E)))

    in_maps = []
    for c in range(N_CORES):
        sl = hs[c * T_CORE:(c + 1) * T_CORE]  # [T_CORE, H]
        h, l = _split_f16(sl)
        in_maps.append({
            "hst_h": _blockify(np.ascontiguousarray(h.T)),
            "hst_l": _blockify(np.ascontiguousarray(l.T)),
            "wt_h": wt_h,
            "wt_l": wt_l,
            "bias_bc": bias_bc,
            "iota_bc": iota_bc,
        })
    return in_maps


def kernel(hidden_states, weight, e_score_correction_bias):
    in_maps = _prepare_inputs(hidden_states, weight, e_score_correction_bias)
    nc = _get_program(1)
    res = run_bass_kernel_spmd(nc, in_maps, list(range(N_CORES)))
    idx = np.concatenate([r["idx_out"] for r in res.results], axis=0)
    w = np.concatenate([r["w_out"] for r in res.results], axis=0)
    return idx.astype(np.int32), w.astype(np.float32)


# revision 3
# speedup vs baseline: 5.4917x; 1.1216x over previous
"""NemotronH Top-k MoE router on 8 Trainium2 NeuronCores.

Token-parallel: 16384 tokens sharded 2048/core; router weight replicated.

Matmul in [t, e] layout: for each 128-token block, PSUM out[t=128, e=256]
accumulates over 32 k-chunks with the hidden chunk as the stationary
operand and the weights as the moving operand.  This puts tokens on PSUM
partitions directly — no PE transposes — and keeps the PE stream dense.

Precision: identical arithmetic to the proven 3-pass fp16 scheme
(hi/lo fp16 planes, x = h + l/2048, 22 mantissa bits; products in fp22,
fp32 PSUM accumulation in the same order), so results are bit-identical
to the baseline kernel that matched the reference exactly.

Routing per block (tokens on partitions, experts on free axis): sigmoid,
bias add, group top-2 via reduce_max + match_replace, group top-4 via
max8 threshold, top-8 via max8 + max_index, per-slot score extraction via
is_equal(iota) + accumulate, normalize, scale.
"""
import sys
sys.path.insert(0, "/opt/trn_rl_repo")

import numpy as np

from concourse import bacc, tile, mybir
from concourse.bass_utils import run_bass_kernel_spmd

F32 = mybir.dt.float32
F16 = mybir.dt.float16
U16 = mybir.dt.uint16
I32 = mybir.dt.int32
AF = mybir.ActivationFunctionType
ALU = mybir.AluOpType

T_TOTAL = 16384
H = 4096
E = 256
G, GS = 8, 32
TOP_K = 8
N_CORES = 8
T_CORE = T_TOTAL // N_CORES      # 2048
TB = 128                         # tokens per block (PSUM partition dim)
NB = T_CORE // TB                # 16 blocks
KC = H // 128                    # 32 k-chunks
KH = KC // 2                     # 16 per weight half (split for startup)
S = 2048.0                       # lo-plane scale (2^11)
ROUTED_SCALING = 2.5


def build_program(reps=1):
    nc = bacc.Bacc("TRN2", target_bir_lowering=False)
    # host pre-layout: [blk, p, c, tb] so each partition line is 8 KB contiguous
    hst_h = nc.dram_tensor("hst_h", [NB, 128, KC, TB], F16, kind="ExternalInput")
    hst_l = nc.dram_tensor("hst_l", [NB, 128, KC, TB], F16, kind="ExternalInput")
    # weights pre-layout [p, c, e], split in two c-halves for faster startup
    wt_h = nc.dram_tensor("wt_h", [128, KC, E], F16, kind="ExternalInput")
    wt_l = nc.dram_tensor("wt_l", [128, KC, E], F16, kind="ExternalInput")
    bias_d = nc.dram_tensor("bias_bc", [128, E], F32, kind="ExternalInput")
    iota_d = nc.dram_tensor("iota_bc", [128, E], F32, kind="ExternalInput")
    idx_out = nc.dram_tensor("idx_out", [T_CORE, TOP_K], I32, kind="ExternalOutput")
    w_out = nc.dram_tensor("w_out", [T_CORE, TOP_K], F32, kind="ExternalOutput")

    with tile.TileContext(nc) as tc:
        with (
            tc.tile_pool(name="const", bufs=1) as cpool,
            tc.tile_pool(name="hs", bufs=3) as hspool,
            tc.tile_pool(name="rt", bufs=2) as rt,
            tc.tile_pool(name="outp", bufs=2) as outp,
            tc.tile_pool(name="psA", bufs=3, space="PSUM") as psA,
            tc.tile_pool(name="psB", bufs=3, space="PSUM") as psB,
        ):
            # weights as two c-halves so first matmuls start after half a DMA
            wh_t = [cpool.tile([128, KH, E], F16, name=f"wh{i}") for i in range(2)]
            wl_t = [cpool.tile([128, KH, E], F16, name=f"wl{i}") for i in range(2)]
            bias_t = cpool.tile([128, E], F32)
            iota_t = cpool.tile([128, E], F32)
            # weights on the ACT HWDGE ring (hidden goes on the SP ring) in
            # quarter-slices so block 0's matmuls can start early
            QW = KH // 2
            for i in range(2):
                for q in range(2):
                    cs = slice(q * QW, (q + 1) * QW)
                    nc.scalar.dma_start(wh_t[i][:, cs, :],
                                        wt_h[:, i * KH + q * QW:i * KH + (q + 1) * QW, :])
            for i in range(2):
                for q in range(2):
                    cs = slice(q * QW, (q + 1) * QW)
                    nc.scalar.dma_start(wl_t[i][:, cs, :],
                                        wt_l[:, i * KH + q * QW:i * KH + (q + 1) * QW, :])
            nc.scalar.dma_start(bias_t[:], bias_d[:])
            nc.scalar.dma_start(iota_t[:], iota_d[:])

            def body():
                for og in range(NB // 8):           # output groups of 8 blocks
                    iouts = outp.tile([128, 8, TOP_K], I32, tag="iouts")
                    wouts = outp.tile([128, 8, TOP_K], F32, tag="wouts")
                    for sub in range(8):
                        blk = og * 8 + sub
                        xh = hspool.tile([128, KC, TB], F16, tag="xh", name=f"xh{blk}")
                        xl = hspool.tile([128, KC, TB], F16, tag="xl", name=f"xl{blk}")
                        nc.sync.dma_start(xh[:], hst_h[blk])
                        nc.sync.dma_start(xl[:], hst_l[blk])

                        main = psA.tile([128, 512], F32, tag="main")
                        corr = psB.tile([128, 512], F32, tag="corr")
                        # main pass: logits_hi[t, e] += xh_c.T @ wh_c
                        for c in range(KC):
                            k, ci = divmod(c, KH)
                            nc.tensor.matmul(
                                main[:, 0:E], xh[:, c, :], wh_t[k][:, ci, :],
                                start=(c == 0), stop=(c == KC - 1))
                        # correction: corr += xh_c.T @ wl_c + xl_c.T @ wh_c
                        # (same interleaved accumulation order as the baseline)
                        for c in range(KC):
                            k, ci = divmod(c, KH)
                            nc.tensor.matmul(
                                corr[:, 0:E], xh[:, c, :], wl_t[k][:, ci, :],
                                start=(c == 0), stop=False)
                            nc.tensor.matmul(
                                corr[:, 0:E], xl[:, c, :], wh_t[k][:, ci, :],
                                start=False, stop=(c == KC - 1))

                        # comb = main + corr/S (exact: 1/S is a power of two)
                        corr_s = rt.tile([128, E], F32, tag="corr_s")
                        nc.scalar.activation(corr_s[:], corr[:, 0:E], AF.Copy,
                                             scale=1.0 / S)
                        comb = rt.tile([128, E], F32, tag="comb")
                        nc.vector.tensor_tensor(comb[:], corr_s[:], main[:, 0:E],
                                                ALU.add)

                        scores = rt.tile([128, E], F32, tag="scores")
                        nc.scalar.activation(scores[:], comb[:], AF.Sigmoid)

                        s4c = rt.tile([128, E], F32, tag="s4c")
                        nc.gpsimd.tensor_tensor(s4c[:], scores[:], bias_t[:], ALU.add)

                        m1 = rt.tile([128, G], F32, tag="m1")
                        nc.vector.reduce_max(
                            m1[:], s4c[:].rearrange("p (g s) -> p g s", g=G),
                            axis=mybir.AxisListType.X)
                        s4cr = rt.tile([128, E], F32, tag="s4cr")
                        nc.vector.match_replace(s4cr[:], m1[:], s4c[:], -1e30)
                        m2 = rt.tile([128, G], F32, tag="m2")
                        nc.vector.reduce_max(
                            m2[:], s4cr[:].rearrange("p (g s) -> p g s", g=G),
                            axis=mybir.AxisListType.X)
                        gsc = rt.tile([128, G], F32, tag="gsc")
                        nc.vector.tensor_tensor(gsc[:], m1[:], m2[:], ALU.add)

                        gsorted = rt.tile([128, 8], F32, tag="gsorted")
                        nc.vector.max(gsorted[:], gsc[:])
                        gmask = rt.tile([128, G], F32, tag="gmask")
                        nc.vector.tensor_scalar(
                            gmask[:], gsc[:], gsorted[:, 3:4], None, ALU.is_ge)

                        masked = rt.tile([128, E], F32, tag="masked")
                        nc.gpsimd.tensor_tensor(
                            masked[:].rearrange("p (g s) -> p g s", g=G),
                            s4c[:].rearrange("p (g s) -> p g s", g=G),
                            gmask[:].unsqueeze(-1).broadcast_to([128, G, GS]),
                            ALU.mult)

                        vals = rt.tile([128, 8], F32, tag="vals")
                        nc.vector.max(vals[:], masked[:])
                        idx16 = rt.tile([128, 8], U16, tag="idx16")
                        nc.vector.max_index(idx16[:], vals[:], masked[:])

                        # per-slot gather scores[idx[k]]: match idx against an
                        # iota row (unique values -> tie-safe), accumulate
                        idxf = rt.tile([128, 8], F32, tag="idxf")
                        nc.vector.tensor_copy(idxf[:], idx16[:])
                        w8 = rt.tile([128, 8], F32, tag="w8")
                        scratch = rt.tile([128, E], F32, tag="scratch")
                        for k in range(TOP_K):
                            nc.vector.scalar_tensor_tensor(
                                scratch[:], iota_t[:], idxf[:, k:k + 1], scores[:],
                                ALU.is_equal, ALU.mult,
                                accum_out=w8[:, k:k + 1])

                        denom = rt.tile([128, 1], F32, tag="denom")
                        nc.vector.reduce_sum(denom[:], w8[:], axis=mybir.AxisListType.X)
                        rec = rt.tile([128, 1], F32, tag="rec")
                        nc.vector.tensor_scalar_add(denom[:], denom[:], 1e-20)
                        nc.vector.reciprocal(rec[:], denom[:])
                        nc.vector.tensor_scalar_mul(rec[:], rec[:], ROUTED_SCALING)

                        nc.vector.tensor_scalar(
                            wouts[:, sub, :], w8[:], rec[:, 0:1], None, ALU.mult)
                        nc.vector.tensor_copy(iouts[:, sub, :], idx16[:])

                    t0 = og * 8 * TB
                    nc.sync.dma_start(
                        idx_out[t0:t0 + 8 * TB, :].rearrange("(s p) k -> p s k", p=128),
                        iouts[:])
                    nc.sync.dma_start(
                        w_out[t0:t0 + 8 * TB, :].rearrange("(s p) k -> p s k", p=128),
                        wouts[:])

            if reps == 1:
                body()
            else:
                with tc.For_i(0, reps, 1):
                    body()
    nc.compile()
    return nc


_PROGRAM_CACHE = {}


def _get_program(reps=1):
    if reps not in _PROGRAM_CACHE:
        _PROGRAM_CACHE[reps] = build_program(reps)
    return _PROGRAM_CACHE[reps]


_F16_MIN_NORMAL = 2.0 ** -14


def _split_f16(x):
    """x (f32) -> (h, l) fp16 planes with x ~= h + l/S; subnormals zeroed."""
    h = x.astype(np.float16)
    h32 = h.astype(np.float32)
    h = np.where(np.abs(h32) < _F16_MIN_NORMAL, np.float16(0), h)
    h32 = h.astype(np.float32)
    l = ((x - h32) * np.float32(S)).astype(np.float16)
    l32 = l.astype(np.float32)
    l = np.where(np.abs(l32) < _F16_MIN_NORMAL, np.float16(0), l)
    return h, l


def _blockify(plane_t):
    """[H, T_CORE] -> [NB, 128, KC, TB] so per-block partition lines are contiguous."""
    # element (h, t): h = c*128 + p, t = blk*TB + tb -> out[blk, p, c, tb]
    a = plane_t.reshape(KC, 128, NB, TB)       # [c, p, blk, tb]
    return np.ascontiguousarray(a.transpose(2, 1, 0, 3))


def _prepare_inputs(hidden_states, weight, e_score_correction_bias):
    hs = np.asarray(hidden_states, dtype=np.float32)
    w = np.asarray(weight, dtype=np.float32)
    b = np.asarray(e_score_correction_bias, dtype=np.float32)

    wh, wl = _split_f16(w)
    # [p, c, e] layout
    wt_h = np.ascontiguousarray(wh.T.reshape(KC, 128, E).transpose(1, 0, 2))
    wt_l = np.ascontiguousarray(wl.T.reshape(KC, 128, E).transpose(1, 0, 2))
    bias_bc = np.ascontiguousarray(np.broadcast_to(b, (128, E)))
    iota_bc = np.ascontiguousarray(
        np.broadcast_to(np.arange(E, dtype=np.float32), (128, E)))

    in_maps = []
    for c in range(N_CORES):
        sl = hs[c * T_CORE:(c + 1) * T_CORE]  # [T_CORE, H]
        h, l = _split_f16(sl)
        in_maps.append({
            "hst_h": _blockify(np.ascontiguousarray(h.T)),
            "hst_l": _blockify(np.ascontiguousarray(l.T)),
            "wt_h": wt_h,
            "wt_l": wt_l,
            "bias_bc": bias_bc,
            "iota_bc": iota_bc,
        })
    return in_maps


def kernel(hidden_states, weight, e_score_correction_bias):
    in_maps = _prepare_inputs(hidden_states, weight, e_score_correction_bias)
    nc = _get_program(1)
    res = run_bass_kernel_spmd(nc, in_maps, list(range(N_CORES)))
    idx = np.concatenate([r["idx_out"] for r in res.results], axis=0)
    w = np.concatenate([r["w_out"] for r in res.results], axis=0)
    return idx.astype(np.int32), w.astype(np.float32)


# revision 4
# speedup vs baseline: 7.0721x; 1.2878x over previous
"""NemotronH Top-k MoE router on 8 Trainium2 NeuronCores.

Token-parallel: 16384 tokens sharded 2048/core; router weight replicated.

Matmul in [t, e] layout: for each 128-token block, PSUM out[t=128, e=256]
accumulates over 32 k-chunks with the hidden chunk as the stationary
operand and the weights as the moving operand.  This puts tokens on PSUM
partitions directly — no PE transposes — and keeps the PE stream dense.

Precision: identical arithmetic to the proven 3-pass fp16 scheme
(hi/lo fp16 planes, x = h + l/2048, 22 mantissa bits; products in fp22,
fp32 PSUM accumulation in the same order), so results are bit-identical
to the baseline kernel that matched the reference exactly.

Routing per block (tokens on partitions, experts on free axis): sigmoid,
bias add, group top-2 via reduce_max + match_replace, group top-4 via
max8 threshold, top-8 via max8 + max_index, per-slot score extraction via
is_equal(iota) + accumulate, normalize, scale.
"""
import sys
sys.path.insert(0, "/opt/trn_rl_repo")

import numpy as np

from concourse import bacc, tile, mybir
from concourse.bass_utils import run_bass_kernel_spmd

F32 = mybir.dt.float32
F16 = mybir.dt.float16
U16 = mybir.dt.uint16
I32 = mybir.dt.int32
AF = mybir.ActivationFunctionType
ALU = mybir.AluOpType

T_TOTAL = 16384
H = 4096
E = 256
G, GS = 8, 32
TOP_K = 8
N_CORES = 8
T_CORE = T_TOTAL // N_CORES      # 2048
TB = 128                         # tokens per block (PSUM partition dim)
NB = T_CORE // TB                # 16 blocks
KC = H // 128                    # 32 k-chunks
KH = KC // 2                     # 16 per weight half (split for startup)
S = 2048.0                       # lo-plane scale (2^11)
ROUTED_SCALING = 2.5


def build_program(reps=1):
    nc = bacc.Bacc("TRN2", target_bir_lowering=False)
    # host pre-layout: [blk, p, c, tb] so each partition line is 8 KB contiguous
    hst_c = nc.dram_tensor("hst_c", [NB, 128, KC, 2 * TB], F16, kind="ExternalInput")
    # weights pre-layout [p, c, e], split in two c-halves for faster startup
    wt_h = nc.dram_tensor("wt_h", [128, KC, E], F16, kind="ExternalInput")
    wt_l = nc.dram_tensor("wt_l", [128, KC, E], F16, kind="ExternalInput")
    bias_d = nc.dram_tensor("bias_bc", [128, E], F32, kind="ExternalInput")
    iota_d = nc.dram_tensor("iota_bc", [128, E], F32, kind="ExternalInput")
    idx_out = nc.dram_tensor("idx_out", [T_CORE, TOP_K], I32, kind="ExternalOutput")
    w_out = nc.dram_tensor("w_out", [T_CORE, TOP_K], F32, kind="ExternalOutput")

    with tile.TileContext(nc) as tc:
        with (
            tc.tile_pool(name="const", bufs=1) as cpool,
            tc.tile_pool(name="hs", bufs=3) as hspool,
            tc.tile_pool(name="rt", bufs=2) as rt,
            tc.tile_pool(name="outp", bufs=2) as outp,
            tc.tile_pool(name="psA", bufs=3, space="PSUM") as psA,
            tc.tile_pool(name="psB", bufs=3, space="PSUM") as psB,
        ):
            # weights as two c-halves so first matmuls start after half a DMA
            wh_t = [cpool.tile([128, KH, E], F16, name=f"wh{i}") for i in range(2)]
            wl_t = [cpool.tile([128, KH, E], F16, name=f"wl{i}") for i in range(2)]
            bias_t = cpool.tile([128, E], F32)
            iota_t = cpool.tile([128, E], F32)
            # weights on the ACT HWDGE ring (hidden goes on the SP ring) in
            # quarter-slices so block 0's matmuls can start early
            QW = KH // 2
            for i in range(2):
                for q in range(2):
                    cs = slice(q * QW, (q + 1) * QW)
                    nc.scalar.dma_start(wh_t[i][:, cs, :],
                                        wt_h[:, i * KH + q * QW:i * KH + (q + 1) * QW, :])
            for i in range(2):
                for q in range(2):
                    cs = slice(q * QW, (q + 1) * QW)
                    nc.scalar.dma_start(wl_t[i][:, cs, :],
                                        wt_l[:, i * KH + q * QW:i * KH + (q + 1) * QW, :])
            nc.scalar.dma_start(bias_t[:], bias_d[:])
            nc.scalar.dma_start(iota_t[:], iota_d[:])

            def body():
                for og in range(NB // 8):           # output groups of 8 blocks
                    iouts = outp.tile([128, 8, TOP_K], I32, tag="iouts")
                    wouts = outp.tile([128, 8, TOP_K], F32, tag="wouts")
                    for sub in range(8):
                        blk = og * 8 + sub
                        xc = hspool.tile([128, KC, 2 * TB], F16, tag="xc", name=f"xc{blk}")
                        nc.sync.dma_start(xc[:], hst_c[blk])
                        xh = xc[:, :, 0:TB]
                        xl = xc[:, :, TB:2 * TB]

                        main = psA.tile([128, 512], F32, tag="main")
                        corr = psB.tile([128, 512], F32, tag="corr")
                        # main pass: logits_hi[t, e] += xh_c.T @ wh_c
                        for c in range(KC):
                            k, ci = divmod(c, KH)
                            nc.tensor.matmul(
                                main[:, 0:E], xh[:, c, :], wh_t[k][:, ci, :],
                                start=(c == 0), stop=(c == KC - 1))
                        # correction: corr += xh_c.T @ wl_c + xl_c.T @ wh_c
                        # (same interleaved accumulation order as the baseline)
                        for c in range(KC):
                            k, ci = divmod(c, KH)
                            nc.tensor.matmul(
                                corr[:, 0:E], xh[:, c, :], wl_t[k][:, ci, :],
                                start=(c == 0), stop=False)
                            nc.tensor.matmul(
                                corr[:, 0:E], xl[:, c, :], wh_t[k][:, ci, :],
                                start=False, stop=(c == KC - 1))

                        # comb = main + corr/S (exact: 1/S is a power of two)
                        corr_s = rt.tile([128, E], F32, tag="corr_s")
                        nc.scalar.activation(corr_s[:], corr[:, 0:E], AF.Copy,
                                             scale=1.0 / S)
                        comb = rt.tile([128, E], F32, tag="comb")
                        nc.vector.tensor_tensor(comb[:], corr_s[:], main[:, 0:E],
                                                ALU.add)

                        scores = rt.tile([128, E], F32, tag="scores")
                        nc.scalar.activation(scores[:], comb[:], AF.Sigmoid)

                        s4c = rt.tile([128, E], F32, tag="s4c")
                        nc.gpsimd.tensor_tensor(s4c[:], scores[:], bias_t[:], ALU.add)

                        m1 = rt.tile([128, G], F32, tag="m1")
                        nc.vector.reduce_max(
                            m1[:], s4c[:].rearrange("p (g s) -> p g s", g=G),
                            axis=mybir.AxisListType.X)
                        s4cr = rt.tile([128, E], F32, tag="s4cr")
                        nc.vector.match_replace(s4cr[:], m1[:], s4c[:], -1e30)
                        m2 = rt.tile([128, G], F32, tag="m2")
                        nc.vector.reduce_max(
                            m2[:], s4cr[:].rearrange("p (g s) -> p g s", g=G),
                            axis=mybir.AxisListType.X)
                        gsc = rt.tile([128, G], F32, tag="gsc")
                        nc.vector.tensor_tensor(gsc[:], m1[:], m2[:], ALU.add)

                        gsorted = rt.tile([128, 8], F32, tag="gsorted")
                        nc.vector.max(gsorted[:], gsc[:])
                        gmask = rt.tile([128, G], F32, tag="gmask")
                        nc.vector.tensor_scalar(
                            gmask[:], gsc[:], gsorted[:, 3:4], None, ALU.is_ge)

                        masked = rt.tile([128, E], F32, tag="masked")
                        nc.gpsimd.tensor_tensor(
                            masked[:].rearrange("p (g s) -> p g s", g=G),
                            s4c[:].rearrange("p (g s) -> p g s", g=G),
                            gmask[:].unsqueeze(-1).broadcast_to([128, G, GS]),
                            ALU.mult)

                        vals = rt.tile([128, 8], F32, tag="vals")
                        nc.vector.max(vals[:], masked[:])
                        idx16 = rt.tile([128, 8], U16, tag="idx16")
                        nc.vector.max_index(idx16[:], vals[:], masked[:])

                        # per-slot gather scores[idx[k]]: match idx against an
                        # iota row (unique values -> tie-safe), accumulate
                        idxf = rt.tile([128, 8], F32, tag="idxf")
                        nc.vector.tensor_copy(idxf[:], idx16[:])
                        w8 = rt.tile([128, 8], F32, tag="w8")
                        scratch = rt.tile([128, E], F32, tag="scratch")
                        for k in range(TOP_K):
                            nc.vector.scalar_tensor_tensor(
                                scratch[:], iota_t[:], idxf[:, k:k + 1], scores[:],
                                ALU.is_equal, ALU.mult,
                                accum_out=w8[:, k:k + 1])

                        denom = rt.tile([128, 1], F32, tag="denom")
                        nc.vector.reduce_sum(denom[:], w8[:], axis=mybir.AxisListType.X)
                        rec = rt.tile([128, 1], F32, tag="rec")
                        nc.vector.tensor_scalar_add(denom[:], denom[:], 1e-20)
                        nc.vector.reciprocal(rec[:], denom[:])
                        nc.vector.tensor_scalar_mul(rec[:], rec[:], ROUTED_SCALING)

                        nc.vector.tensor_scalar(
                            wouts[:, sub, :], w8[:], rec[:, 0:1], None, ALU.mult)
                        nc.vector.tensor_copy(iouts[:, sub, :], idx16[:])

                    t0 = og * 8 * TB
                    nc.sync.dma_start(
                        idx_out[t0:t0 + 8 * TB, :].rearrange("(s p) k -> p s k", p=128),
                        iouts[:])
                    nc.sync.dma_start(
                        w_out[t0:t0 + 8 * TB, :].rearrange("(s p) k -> p s k", p=128),
                        wouts[:])

            if reps == 1:
                body()
            else:
                with tc.For_i(0, reps, 1):
                    body()
    nc.compile()
    return nc


_PROGRAM_CACHE = {}


def _get_program(reps=1):
    if reps not in _PROGRAM_CACHE:
        _PROGRAM_CACHE[reps] = build_program(reps)
    return _PROGRAM_CACHE[reps]


_F16_MIN_NORMAL = 2.0 ** -14


def _split_f16(x):
    """x (f32) -> (h, l) fp16 planes with x ~= h + l/S; subnormals zeroed."""
    h = x.astype(np.float16)
    h32 = h.astype(np.float32)
    h = np.where(np.abs(h32) < _F16_MIN_NORMAL, np.float16(0), h)
    h32 = h.astype(np.float32)
    l = ((x - h32) * np.float32(S)).astype(np.float16)
    l32 = l.astype(np.float32)
    l = np.where(np.abs(l32) < _F16_MIN_NORMAL, np.float16(0), l)
    return h, l


def _blockify(plane_t):
    """[H, T_CORE] -> [NB, 128, KC, TB] so per-block partition lines are contiguous."""
    # element (h, t): h = c*128 + p, t = blk*TB + tb -> out[blk, p, c, tb]
    a = plane_t.reshape(KC, 128, NB, TB)       # [c, p, blk, tb]
    return np.ascontiguousarray(a.transpose(2, 1, 0, 3))


def _prepare_inputs(hidden_states, weight, e_score_correction_bias):
    hs = np.asarray(hidden_states, dtype=np.float32)
    w = np.asarray(weight, dtype=np.float32)
    b = np.asarray(e_score_correction_bias, dtype=np.float32)

    wh, wl = _split_f16(w)
    # [p, c, e] layout
    wt_h = np.ascontiguousarray(wh.T.reshape(KC, 128, E).transpose(1, 0, 2))
    wt_l = np.ascontiguousarray(wl.T.reshape(KC, 128, E).transpose(1, 0, 2))
    bias_bc = np.ascontiguousarray(np.broadcast_to(b, (128, E)))
    iota_bc = np.ascontiguousarray(
        np.broadcast_to(np.arange(E, dtype=np.float32), (128, E)))

    in_maps = []
    for c in range(N_CORES):
        sl = hs[c * T_CORE:(c + 1) * T_CORE]  # [T_CORE, H]
        h, l = _split_f16(sl)
        bh = _blockify(np.ascontiguousarray(h.T))
        bl = _blockify(np.ascontiguousarray(l.T))
        in_maps.append({
            "hst_c": np.ascontiguousarray(np.concatenate([bh, bl], axis=3)),
            "wt_h": wt_h,
            "wt_l": wt_l,
            "bias_bc": bias_bc,
            "iota_bc": iota_bc,
        })
    return in_maps


def kernel(hidden_states, weight, e_score_correction_bias):
    in_maps = _prepare_inputs(hidden_states, weight, e_score_correction_bias)
    nc = _get_program(1)
    res = run_bass_kernel_spmd(nc, in_maps, list(range(N_CORES)))
    idx = np.concatenate([r["idx_out"] for r in res.results], axis=0)
    w = np.concatenate([r["w_out"] for r in res.results], axis=0)
    return idx.astype(np.int32), w.astype(np.float32)
